# revision 2
# baseline (speedup 1.0000x reference)
"""MoE expert FFN (swiglu) kernel for 8 trn2 NeuronCores.

Expert parallelism: 8 experts, one per core. Each core computes, for its
expert e:
    h   = x_e @ w1_e            # [2048, 2048] @ [2048, 2816]
    act = silu(h[:, :1408]) * h[:, 1408:]
    out = act @ w2_e            # [2048, 1408] @ [1408, 2048]

Tokens arrive pre-sorted by expert with equal counts (2048/expert), so
sharding is a static slice and the gather is a concat. No collectives.

Device-side layout (all bf16 compute, fp32 PSUM accumulation, bf16 out):
  mm1: out[f, t] tiles; lhsT = w1 128x128 tiles (stationary),
       rhs = xT[h, t] (moving, N=512) -> inter is [f, t], the layout mm2
       needs, so no on-device transpose anywhere (x is transposed on host).
  swiglu pairs: psum tile f-block j (a) with f-block j+11 (b);
       act_j = silu(a) * b  via ACT(Silu) + DVE mul -> bf16 SBUF.
  mm2: out[t, h] tiles; lhsT = act[f, t] 128-col slices (stationary),
       rhs = w2[f, h] (moving, N=512). PSUM -> SBUF bf16 -> DMA to out.

v2 vs v1 (516us): kernel is PE-bound (~450us of bf16 matmul); v1 lost
~53us of PE idle to DMA ordering. Fixes:
  - w1 is prepacked on host into swiglu-paired column slabs
    [p, pair j, k, {a|b}, 128] so one contiguous DMA delivers BOTH halves
    of a pair for all k. v1's whole-row [128, 2816] tiles made pair 0
    wait ~27us for the b-half columns of all 16 k-tiles.
  - w1 slabs are split into 4 k-group DMAs each, alternated across the
    two HWDGE queues; first matmul starts after ~0.6MB instead of ~12us.
  - x0 k=0..3 ride the HWDGE queues ahead of w1 (SWDGE starts slowly).
  - out is stored as bf16 (host upcasts): halves store traffic and the
    ~14us post-matmul DMA drain. rel-err ~4.5e-3, well under 2e-2.
"""

import os
import sys

sys.path.insert(0, "/opt/trn_rl_repo")

import numpy as np
import ml_dtypes

E = 8             # experts == cores
T_TOTAL = 16384
H = 2048
F = 1408
F2 = 2 * F        # 2816
TPC = T_TOTAL // E  # 2048 tokens per core
CHUNK = 512
NCH = TPC // CHUNK          # 4 chunks
KH = H // 128               # 16 contraction tiles for mm1
NF = F // 128               # 11 f-blocks per half (a / b)
NT = CHUNK // 128           # 4 m-tiles per chunk in mm2
NHO = H // 512              # 4 output column blocks
KG = 4                      # w1 k-groups per pair slab (4 k-tiles each)
GW = (KH // KG) * 256       # 1024 cols per k-group slab

_CACHE = {}

# Optional knobs read by test.py (not used by the grading harness).
TRACE = os.environ.get("BASS_TRACE_KERNEL", "0") == "1"
LAST = {}


def _build():
    from concourse import bacc, tile, mybir

    bf16 = mybir.dt.bfloat16
    f32 = mybir.dt.float32
    SILU = mybir.ActivationFunctionType.Silu

    # Bacc (not plain Bass): its lowering pipeline splits multi-sem waits
    # into EventSemaphore pairs — TRN2 allows at most 1 wait per instruction.
    nc = bacc.Bacc()
    xT_d = nc.declare_dram_parameter("xT", [H, TPC], bf16, isOutput=False)
    # w1 prepacked on host: [128, 11 pairs * 16 k * (a|b) * 128] — see kernel().
    w1_d = nc.declare_dram_parameter("w1", [128, NF * KH * 256], bf16, isOutput=False)
    w2_d = nc.declare_dram_parameter("w2", [F, H], bf16, isOutput=False)
    out_d = nc.declare_dram_parameter("out", [TPC, H], bf16, isOutput=True)

    with tile.TileContext(nc) as tc:
        with (
            tc.tile_pool(name="w1p", bufs=1) as w1p,
            tc.tile_pool(name="w2p", bufs=1) as w2p,
            tc.tile_pool(name="xp", bufs=2) as xp,
            tc.tile_pool(name="actp", bufs=1) as actp,
            tc.tile_pool(name="tmpp", bufs=2) as tmpp,
            tc.tile_pool(name="outp", bufs=4) as outp,
            tc.tile_pool(name="psp", bufs=8, space="PSUM") as psp,
        ):
            # x chunk 0, k=0..3 on the HWDGE queues ahead of everything —
            # the first matmul needs x0[0] and SWDGE has a slow first fetch.
            x0_t = []
            for k in range(KH):
                t = xp.tile([128, CHUNK], bf16, tag=f"x_{k}", name=f"x0_{k}")
                x0_t.append(t)
            for k in range(4):
                eng = [nc.sync, nc.scalar, nc.scalar, nc.scalar][k]
                eng.dma_start(out=x0_t[k][:], in_=xT_d[k * 128 : (k + 1) * 128, 0:CHUNK])

            # Resident w1, one [128, 4*256] slab per (pair j, k-group g),
            # alternating the two HWDGE queues. Delivery order matches the
            # PE's consumption order (pair 0 first, both swiglu halves).
            w1_t = [[None] * (KH // KG) for _ in range(NF)]
            for j in range(NF):
                for g in range(KH // KG):
                    t = w1p.tile([128, GW], bf16, tag=f"w1_{j}_{g}")
                    w1_t[j][g] = t
                    eng = nc.sync if (j * (KH // KG) + g) % 2 == 0 else nc.scalar
                    c0 = j * (KH * 256) + g * GW
                    eng.dma_start(out=t[:], in_=w1_d[:, c0 : c0 + GW])

            # Rest of x chunk 0 on gpsimd (SWDGE) so the HWDGE queues are
            # free to stream w1.
            for k in range(4, KH):
                nc.gpsimd.dma_start(
                    out=x0_t[k][:], in_=xT_d[k * 128 : (k + 1) * 128, 0:CHUNK]
                )

            # Resident w2: 11 tiles [128, 2048]; not needed until mm2 of
            # chunk 0 (~120us in), so queue behind w1 on ACT.
            w2_t = []
            for k in range(NF):
                t = w2p.tile([128, H], bf16, tag=f"w2_{k}")
                w2_t.append(t)
                nc.scalar.dma_start(out=t[:], in_=w2_d[k * 128 : (k + 1) * 128, :])

            for c in range(NCH):
                # Stream this chunk of tokens (columns of xT); chunk 0 was
                # preloaded above. gpsimd keeps the HWDGE queues clear.
                if c == 0:
                    x_t = x0_t
                else:
                    x_t = []
                    for k in range(KH):
                        t = xp.tile([128, CHUNK], bf16, tag=f"x_{k}", name=f"x_{c}_{k}")
                        x_t.append(t)
                        nc.gpsimd.dma_start(
                            out=t[:],
                            in_=xT_d[k * 128 : (k + 1) * 128,
                                     c * CHUNK : (c + 1) * CHUNK],
                        )

                # mm1 + swiglu, one (a, b) f-block pair at a time.
                act_t = []
                for j in range(NF):
                    ps_a = psp.tile([128, CHUNK], f32, tag="ps")
                    ps_b = psp.tile([128, CHUNK], f32, tag="ps")
                    for k in range(KH):
                        o = (k % KG) * 256
                        nc.tensor.matmul(
                            ps_a[:],
                            w1_t[j][k // KG][:, o : o + 128],
                            x_t[k][:],
                            start=(k == 0),
                            stop=(k == KH - 1),
                        )
                    for k in range(KH):
                        o = (k % KG) * 256
                        nc.tensor.matmul(
                            ps_b[:],
                            w1_t[j][k // KG][:, o + 128 : o + 256],
                            x_t[k][:],
                            start=(k == 0),
                            stop=(k == KH - 1),
                        )
                    tmp = tmpp.tile([128, CHUNK], f32, tag="tmp")
                    nc.scalar.activation(tmp[:], ps_a[:], SILU)
                    a = actp.tile([128, CHUNK], bf16, tag=f"act_{j}")
                    act_t.append(a)
                    nc.vector.tensor_mul(a[:], tmp[:], ps_b[:])

                # mm2: out[t, h] for this chunk.
                for m in range(NT):
                    po = [
                        psp.tile([128, 512], f32, tag="ps", name=f"po_{c}_{m}_{n}")
                        for n in range(NHO)
                    ]
                    for k in range(NF):
                        lhsT = act_t[k][:, m * 128 : (m + 1) * 128]
                        for n in range(NHO):
                            nc.tensor.matmul(
                                po[n][:],
                                lhsT,
                                w2_t[k][:, n * 512 : (n + 1) * 512],
                                start=(k == 0),
                                stop=(k == NF - 1),
                            )
                    r0 = c * CHUNK + m * 128
                    for n in range(NHO):
                        osb = outp.tile([128, 512], bf16, tag="osb")
                        nc.scalar.copy(osb[:], po[n][:])
                        nc.sync.dma_start(
                            out=out_d[r0 : r0 + 128, n * 512 : (n + 1) * 512],
                            in_=osb[:],
                        )
    if not nc.is_finalized():
        nc.finalize()  # Bacc.finalize runs the lowering pipeline (sem split, alloc_regs)
    return nc


def _get_nc():
    if "nc" not in _CACHE:
        _CACHE["nc"] = _build()
    return _CACHE["nc"]


def _pack_w1(w1e):
    """[H, 2F] f32 -> [128, 11*16*2*128] bf16, swiglu-paired column slabs.

    Layout: col index = ((j * KH + k) * 2 + half) * 128 + c, holding
    w1e[k*128 + p, (j + half*NF) * 128 + c] at partition row p.
    """
    a = w1e.reshape(KH, 128, 2 * NF, 128)            # k, p, fb, c
    t = a.transpose(1, 2, 0, 3)                      # p, fb, k, c
    pairs = np.stack([t[:, :NF], t[:, NF:]], axis=3)  # p, j, k, half, c
    return np.ascontiguousarray(
        pairs.reshape(128, NF * KH * 256).astype(ml_dtypes.bfloat16)
    )


def kernel(permuted_hidden_states, num_tokens_per_expert, w1, w2):
    from concourse.bass_utils import run_bass_kernel_spmd

    x = np.asarray(permuted_hidden_states, dtype=np.float32)
    w1 = np.asarray(w1, dtype=np.float32)
    w2 = np.asarray(w2, dtype=np.float32)
    ntpe = np.asarray(num_tokens_per_expert)
    assert x.shape == (T_TOTAL, H) and w1.shape == (E, H, F2) and w2.shape == (E, F, H)
    # Reference semantics rely on the static equal split.
    assert np.all(ntpe == TPC), f"expected equal {TPC}-token splits, got {ntpe}"

    bf = ml_dtypes.bfloat16
    in_maps = []
    for e in range(E):
        xe = x[e * TPC : (e + 1) * TPC]
        in_maps.append(
            {
                "xT": np.ascontiguousarray(xe.T).astype(bf),
                "w1": _pack_w1(w1[e]),
                "w2": np.ascontiguousarray(w2[e]).astype(bf),
            }
        )

    nc = _get_nc()
    res = run_bass_kernel_spmd(nc, in_maps, list(range(E)), trace=TRACE)
    LAST["exec_time_ns"] = res.exec_time_ns
    LAST["mean_exec_time_ns"] = res.mean_exec_time_ns
    LAST["profile_json"] = res.profile_json
    out = np.concatenate(
        [np.asarray(res.results[i]["out"], dtype=np.float32) for i in range(E)], axis=0
    )
    return np.ascontiguousarray(out)


# revision 4
# speedup vs baseline: 1.0551x; 1.0551x over previous
"""MoE expert FFN (swiglu) kernel for 8 trn2 NeuronCores.

Expert parallelism: 8 experts, one per core. Each core computes, for its
expert e:
    h   = x_e @ w1_e            # [2048, 2048] @ [2048, 2816]
    act = silu(h[:, :1408]) * h[:, 1408:]
    out = act @ w2_e            # [2048, 1408] @ [1408, 2048]

Tokens arrive pre-sorted by expert with equal counts (2048/expert), so
sharding is a static slice and the gather is a concat. No collectives.

Device-side layout (all bf16 compute, fp32 PSUM accumulation, bf16 out):
  mm1: out[f, t] tiles; lhsT = w1 128x128 tiles (stationary),
       rhs = xT[h, t] (moving, N=512) -> inter is [f, t], the layout mm2
       needs, so no on-device transpose anywhere (x is transposed on host).
  swiglu pairs: psum tile f-block j (a) with f-block j+11 (b);
       act_j = silu(a) * b  via ACT(Silu) + DVE mul -> bf16 SBUF.
  mm2: out[t, h] tiles; lhsT = act[f, t] 128-col slices (stationary),
       rhs = w2[f, h] (moving, N=512). PSUM -> SBUF bf16 -> DMA to out.

v2 vs v1 (516us): kernel is PE-bound (~450us of bf16 matmul); v1 lost
~53us of PE idle to DMA ordering. Fixes:
  - w1 is prepacked on host into swiglu-paired column slabs
    [p, pair j, k, {a|b}, 128] so one contiguous DMA delivers BOTH halves
    of a pair for all k. v1's whole-row [128, 2816] tiles made pair 0
    wait ~27us for the b-half columns of all 16 k-tiles.
  - w1 slabs are split into 4 k-group DMAs each, alternated across the
    two HWDGE queues; first matmul starts after ~0.6MB instead of ~12us.
  - x0 k=0..3 ride the HWDGE queues ahead of w1 (SWDGE starts slowly).
  - out is stored as bf16 (host upcasts): halves store traffic and the
    ~14us post-matmul DMA drain. rel-err ~4.5e-3, well under 2e-2.
"""

import os
import sys

sys.path.insert(0, "/opt/trn_rl_repo")

import numpy as np
import ml_dtypes

E = 8             # experts == cores
T_TOTAL = 16384
H = 2048
F = 1408
F2 = 2 * F        # 2816
TPC = T_TOTAL // E  # 2048 tokens per core
CHUNK = 512
NCH = TPC // CHUNK          # 4 chunks
KH = H // 128               # 16 contraction tiles for mm1
NF = F // 128               # 11 f-blocks per half (a / b)
NT = CHUNK // 128           # 4 m-tiles per chunk in mm2
NHO = H // 512              # 4 output column blocks
KG = 4                      # w1 k-groups per pair slab (4 k-tiles each)
GW = (KH // KG) * 256       # 1024 cols per k-group slab

_CACHE = {}

# Optional knobs read by test.py (not used by the grading harness).
TRACE = os.environ.get("BASS_TRACE_KERNEL", "0") == "1"
LAST = {}


def _build():
    from concourse import bacc, tile, mybir

    bf16 = mybir.dt.bfloat16
    f32 = mybir.dt.float32
    SILU = mybir.ActivationFunctionType.Silu

    # Bacc (not plain Bass): its lowering pipeline splits multi-sem waits
    # into EventSemaphore pairs — TRN2 allows at most 1 wait per instruction.
    nc = bacc.Bacc()
    xT_d = nc.declare_dram_parameter("xT", [H, TPC], bf16, isOutput=False)
    # w1 prepacked on host: [128, 11 pairs * 16 k * (a|b) * 128] — see kernel().
    w1_d = nc.declare_dram_parameter("w1", [128, NF * KH * 256], bf16, isOutput=False)
    w2_d = nc.declare_dram_parameter("w2", [F, H], bf16, isOutput=False)
    out_d = nc.declare_dram_parameter("out", [TPC, H], bf16, isOutput=True)

    with tile.TileContext(nc) as tc:
        with (
            tc.tile_pool(name="w1p", bufs=1) as w1p,
            tc.tile_pool(name="w2p", bufs=1) as w2p,
            tc.tile_pool(name="xp", bufs=2) as xp,
            tc.tile_pool(name="actp", bufs=1) as actp,
            tc.tile_pool(name="tmpp", bufs=2) as tmpp,
            tc.tile_pool(name="outp", bufs=4) as outp,
            tc.tile_pool(name="psp", bufs=8, space="PSUM") as psp,
        ):
            # DMA issue blocks the issuing ENGINE (~0.6us/DMA + queue
            # backpressure for the whole transfer backlog), so engines with
            # compute roles must stay clear of bulk loads:
            #   sync (HWDGE):   w1-even slabs, then w2, then out stores.
            #   scalar (HWDGE): 4 tiny x0 head tiles ONLY — ACT must be free
            #                   by ~16us or silu(pair0) blocks the PSUM-bank
            #                   rotation at pair 4 (v2 lost 19us to this).
            #   gpsimd (SWDGE): x0 tail, w1-odd, x chunks 1-3. FIFO order
            #                   keeps w1 ahead of the deferrable x chunks.
            #   vector: no DMA — its muls also gate PSUM reuse.
            x0_t = []
            for k in range(KH):
                t = xp.tile([128, CHUNK], bf16, tag=f"x_{k}", name=f"x0_{k}")
                x0_t.append(t)

            w1_t = [[None] * (KH // KG) for _ in range(NF)]
            for j in range(NF):
                for g in range(KH // KG):
                    w1_t[j][g] = w1p.tile(
                        [128, GW], bf16, tag=f"w1_{j}_{g}", name=f"w1_{j}_{g}"
                    )

            # First matmul needs w1(0,0) and x0[0] — issue them first, on
            # different queues so they transfer in parallel.
            nc.sync.dma_start(out=w1_t[0][0][:], in_=w1_d[:, 0:GW])
            for k in range(4):
                nc.scalar.dma_start(
                    out=x0_t[k][:], in_=xT_d[k * 128 : (k + 1) * 128, 0:CHUNK]
                )
            for k in range(4, KH):
                nc.gpsimd.dma_start(
                    out=x0_t[k][:], in_=xT_d[k * 128 : (k + 1) * 128, 0:CHUNK]
                )
            for j in range(NF):
                for g in range(KH // KG):
                    if j == 0 and g == 0:
                        continue
                    eng = nc.sync if (j * (KH // KG) + g) % 2 == 0 else nc.gpsimd
                    c0 = j * (KH * 256) + g * GW
                    eng.dma_start(out=w1_t[j][g][:], in_=w1_d[:, c0 : c0 + GW])

            # Resident w2: 11 tiles [128, 2048]; not needed until mm2 of
            # chunk 0 (~120us in) — behind w1-even on the sync queue.
            w2_t = []
            for k in range(NF):
                t = w2p.tile([128, H], bf16, tag=f"w2_{k}")
                w2_t.append(t)
                nc.sync.dma_start(out=t[:], in_=w2_d[k * 128 : (k + 1) * 128, :])

            for c in range(NCH):
                # Stream this chunk of tokens (columns of xT); chunk 0 was
                # preloaded above. gpsimd keeps the HWDGE queues clear.
                if c == 0:
                    x_t = x0_t
                else:
                    x_t = []
                    for k in range(KH):
                        t = xp.tile([128, CHUNK], bf16, tag=f"x_{k}", name=f"x_{c}_{k}")
                        x_t.append(t)
                        nc.gpsimd.dma_start(
                            out=t[:],
                            in_=xT_d[k * 128 : (k + 1) * 128,
                                     c * CHUNK : (c + 1) * CHUNK],
                        )

                # mm1 + swiglu, one (a, b) f-block pair at a time.
                act_t = []
                for j in range(NF):
                    ps_a = psp.tile([128, CHUNK], f32, tag="ps")
                    ps_b = psp.tile([128, CHUNK], f32, tag="ps")
                    for k in range(KH):
                        o = (k % KG) * 256
                        nc.tensor.matmul(
                            ps_a[:],
                            w1_t[j][k // KG][:, o : o + 128],
                            x_t[k][:],
                            start=(k == 0),
                            stop=(k == KH - 1),
                        )
                    for k in range(KH):
                        o = (k % KG) * 256
                        nc.tensor.matmul(
                            ps_b[:],
                            w1_t[j][k // KG][:, o + 128 : o + 256],
                            x_t[k][:],
                            start=(k == 0),
                            stop=(k == KH - 1),
                        )
                    tmp = tmpp.tile([128, CHUNK], f32, tag="tmp")
                    nc.scalar.activation(tmp[:], ps_a[:], SILU)
                    a = actp.tile([128, CHUNK], bf16, tag=f"act_{j}")
                    act_t.append(a)
                    nc.vector.tensor_mul(a[:], tmp[:], ps_b[:])

                # mm2: out[t, h] for this chunk.
                for m in range(NT):
                    po = [
                        psp.tile([128, 512], f32, tag="ps", name=f"po_{c}_{m}_{n}")
                        for n in range(NHO)
                    ]
                    for k in range(NF):
                        lhsT = act_t[k][:, m * 128 : (m + 1) * 128]
                        for n in range(NHO):
                            nc.tensor.matmul(
                                po[n][:],
                                lhsT,
                                w2_t[k][:, n * 512 : (n + 1) * 512],
                                start=(k == 0),
                                stop=(k == NF - 1),
                            )
                    r0 = c * CHUNK + m * 128
                    for n in range(NHO):
                        osb = outp.tile([128, 512], bf16, tag="osb")
                        nc.scalar.copy(osb[:], po[n][:])
                        nc.sync.dma_start(
                            out=out_d[r0 : r0 + 128, n * 512 : (n + 1) * 512],
                            in_=osb[:],
                        )
    if not nc.is_finalized():
        nc.finalize()  # Bacc.finalize runs the lowering pipeline (sem split, alloc_regs)
    return nc


def _get_nc():
    if "nc" not in _CACHE:
        _CACHE["nc"] = _build()
    return _CACHE["nc"]


def _pack_w1(w1e):
    """[H, 2F] f32 -> [128, 11*16*2*128] bf16, swiglu-paired column slabs.

    Layout: col index = ((j * KH + k) * 2 + half) * 128 + c, holding
    w1e[k*128 + p, (j + half*NF) * 128 + c] at partition row p.
    """
    a = w1e.reshape(KH, 128, 2 * NF, 128)            # k, p, fb, c
    t = a.transpose(1, 2, 0, 3)                      # p, fb, k, c
    pairs = np.stack([t[:, :NF], t[:, NF:]], axis=3)  # p, j, k, half, c
    return np.ascontiguousarray(
        pairs.reshape(128, NF * KH * 256).astype(ml_dtypes.bfloat16)
    )


def kernel(permuted_hidden_states, num_tokens_per_expert, w1, w2):
    from concourse.bass_utils import run_bass_kernel_spmd

    x = np.asarray(permuted_hidden_states, dtype=np.float32)
    w1 = np.asarray(w1, dtype=np.float32)
    w2 = np.asarray(w2, dtype=np.float32)
    ntpe = np.asarray(num_tokens_per_expert)
    assert x.shape == (T_TOTAL, H) and w1.shape == (E, H, F2) and w2.shape == (E, F, H)
    # Reference semantics rely on the static equal split.
    assert np.all(ntpe == TPC), f"expected equal {TPC}-token splits, got {ntpe}"

    bf = ml_dtypes.bfloat16
    in_maps = []
    for e in range(E):
        xe = x[e * TPC : (e + 1) * TPC]
        in_maps.append(
            {
                "xT": np.ascontiguousarray(xe.T).astype(bf),
                "w1": _pack_w1(w1[e]),
                "w2": np.ascontiguousarray(w2[e]).astype(bf),
            }
        )

    nc = _get_nc()
    res = run_bass_kernel_spmd(nc, in_maps, list(range(E)), trace=TRACE)
    LAST["exec_time_ns"] = res.exec_time_ns
    LAST["mean_exec_time_ns"] = res.mean_exec_time_ns
    LAST["profile_json"] = res.profile_json
    out = np.concatenate(
        [np.asarray(res.results[i]["out"], dtype=np.float32) for i in range(E)], axis=0
    )
    return np.ascontiguousarray(out)
